# revision 9
# baseline (speedup 1.0000x reference)
"""Trainium2 Bass kernel for CustomISTFT (N_FFT=4096, HOP=1024, T=4096 frames).

Per core (frames sharded 512/core across 8 cores):
  Cooley-Tukey split of the 4096-point inverse DFT: k = 64*j1 + res,
  n = m1 + 64*m2.  z is loaded ONCE into SBUF as zfull[(c,j1), res, t]
  (two big k-major DMAs, 1KB descriptors), so stage 1 needs no gathers:
  column c of the length-64 residue DFT is two accumulating matmuls
  (residues res=c and res=64-c) against zfull slices, contraction over
  (channel, j1) = 97 partitions.  Hermitian symmetry gives
  A[m1, 64-c] = conj(A[m1, c]), so only c = 0..32 is computed; the
  conjugate fold, window and normalization are in the stage-2 weights
  (contraction over (re/im, c) = 66).  Both corner turns (m1 <-> c and
  m2 -> OLA layout) are per-slice SBUF->SBUF DMAs into pre-transposed
  tiles; DMA triggers alternate between the two HWDGE queues (sync +
  scalar).  The overlap-add runs on the vector engine in
  [n mod 128, n div 128, t] layout so all shifts are free-dim.  The
  imaginary channel is win[n]*(b0[t] + (-1)^n b2048[t])/4096 (rank-2
  per parity) via K=8 matmuls that also perform its overlap-add; it
  ships as bf16.  z is pre-cast to bf16 on the host.  Host: shard,
  gather, reorder, halo-add between neighbor cores, exact wsum
  correction on the two edge blocks.
"""

import numpy as np
import ml_dtypes

N_FFT = 4096
HOP = 1024
FREQ = 2049
T_FRAMES = 4096
N_CORES = 8
T_CORE = T_FRAMES // N_CORES  # 512
L_FULL = (T_FRAMES - 1) * HOP + N_FFT
OUT_LEN = L_FULL - N_FFT

_bf16 = ml_dtypes.bfloat16


# ---------------------------------------------------------------- weights
def build_weights(window):
    """w1 [97,66,128] bf16: row (c*64+j1) matches zfull partition, col
    2g+half (half 0: res=g, half 1: res=64-g), inner (ri_out, m1).
    w2 [66,64,64] bf16 ([(ri*33+c), m1, (par,nh)-col], conj-fold +
    window/3 folded).  wim [8,1024] bf16."""
    win = window.astype(np.float64)
    mu = np.exp(2j * np.pi / 4096)
    w64c = np.exp(2j * np.pi / 64)
    m1v = np.arange(64)

    w1 = np.zeros((97, 66, 128), dtype=np.float64)
    for g in range(33):
        coef = {}
        for j1 in range(64):
            k = 64 * j1 + g
            e = w64c ** (m1v * j1)
            if k <= 2048:
                coef[(0, k)] = coef.get((0, k), 0) + e
                coef[(1, k)] = coef.get((1, k), 0) + 1j * e
            else:
                kr = 4096 - k
                coef[(0, kr)] = coef.get((0, kr), 0) + e
                coef[(1, kr)] = coef.get((1, kr), 0) - 1j * e
        tw = mu ** (m1v * g)
        for (ch, k), v0 in coef.items():
            v = v0 * tw
            res, j1 = k % 64, k // 64
            if g in (0, 32):
                col = 2 * g
            elif res == g:
                col = 2 * g
            else:
                assert res == 64 - g
                col = 2 * g + 1
            row = ch * 64 + j1
            w1[row, col, :64] += v.real
            w1[row, col, 64:] += v.imag

    # stage 2: x[m1 + 64*m2] = (1/4096) * [ ReA[m1,0] + (-1)^m2 ReA[m1,32]
    #   + sum_{c=1..31} 2*(cos(th) ReA[m1,c] - sin(th) ImA[m1,c]) ],
    # th = 2*pi*m2*c/64, then * win[n]*4096/3.  Column cc = par*32+nh
    # with m2 = 2*nh+par.
    w2 = np.zeros((66, 64, 64), dtype=np.float64)
    cc = np.arange(64)
    m2 = 2 * (cc % 32) + (cc // 32)
    for c in range(33):
        fac = 2.0 if 1 <= c <= 31 else 1.0
        th = 2 * np.pi * m2 * c / 64.0
        for m1 in range(64):
            n = m1 + 64 * m2
            wn = win[n] / 3.0  # win * (4096/3) / 4096
            w2[c, m1, :] = fac * np.cos(th) * wn
            w2[33 + c, m1, :] = -fac * np.sin(th) * wn

    # wim[(2r+par), i] = win[i + 1024 r]/3 * (par == i%2)
    wim = np.zeros((8, 1024), dtype=np.float64)
    iv = np.arange(1024)
    for r in range(4):
        for par in range(2):
            wim[2 * r + par] = (win[iv + 1024 * r] / 3.0) * (iv % 2 == par)
    return w1.astype(_bf16), w2.astype(_bf16), wim.astype(_bf16)


# ---------------------------------------------------------------- device program
def emit_kernel(tc, outre_ap, outim_ap, z_ap, w1_ap, w2_ap, wim_ap, T):
    """Per-core program.  T frames (multiple of 128).
    outre [128, 8, SPAD] f32:  outre[p, ih, s] =
        sum_r win*x[p + 128*ih + 1024*r, s - r]  (real channel, s in [0,T+3))
    outim [SC, 128, 1024] bf16: outim[sc, sp, i] = imag channel at block
        s = 128*sc + sp, position i."""
    import concourse.mybir as mybir
    from contextlib import ExitStack

    nc = tc.nc
    dt = mybir.dt
    f32, bf16 = dt.float32, dt.bfloat16
    SB = T + 3
    SC = (SB + 127) // 128
    SPAD = outre_ap.shape[2]
    assert SPAD >= SB and outim_ap.shape[0] == SC

    with ExitStack() as ctx:
        const = ctx.enter_context(tc.tile_pool(name="const", bufs=1))

        # persistent tiles
        rt = const.tile([66, 64, T], bf16)  # A^T: [(ri*33+c), m1, t]
        sig = const.tile([128, 8, SPAD], f32)
        w2_sb = const.tile([66, 64, 64], bf16)
        wim_sb = const.tile([8, 1024], bf16)
        cve = const.tile([1, T], bf16)
        cvo = const.tile([1, T], bf16)
        b0t = const.tile([1, T], bf16)
        b2t = const.tile([1, T], bf16)
        cs = const.tile([8, SC * 128], bf16)

        nc.scalar.dma_start(w2_sb[:], w2_ap[:])
        nc.scalar.dma_start(wim_sb[:], wim_ap[:])
        nc.any.memset(sig[:], 0.0)
        nc.any.memset(cs[:], 0.0)

        # b0 = z[1,0,:], b2048 = z[1,2048,:];  CS[2r+par, s] = cv_par[s-r]
        nc.sync.dma_start(b0t[:], z_ap[1, 0:1, :])
        nc.sync.dma_start(b2t[:], z_ap[1, 2048:2049, :])
        nc.vector.tensor_add(cve[:], b0t[:], b2t[:])
        nc.vector.tensor_sub(cvo[:], b0t[:], b2t[:])
        for r in range(4):
            nc.sync.dma_start(cs[2 * r : 2 * r + 1, r : r + T], cve[:])
            nc.sync.dma_start(cs[2 * r + 1 : 2 * r + 2, r : r + T], cvo[:])

        # ---- phase 1: z resident in SBUF, 33 stage-1 calls, corner turn
        with (
            tc.tile_pool(name="ph1", bufs=1) as ph1,
            tc.tile_pool(name="s1ps", bufs=3, space="PSUM") as s1ps,
            tc.tile_pool(name="aslot", bufs=4) as apool,
        ):
            # zfull[(c*64+j1), res, t] = z[c, res + 64*j1, t]; rows 33-63
            # are zero pad (weights there are zero), row 32/96 only res=0
            # is real data (k=2048), other res zeroed.
            zfull = ph1.tile([128, 64, T], bf16)
            w1_sb = ph1.tile([97, 66, 128], bf16)
            nc.scalar.dma_start(w1_sb[:], w1_ap[:])
            nc.vector.memset(zfull[32:64, :, :], 0.0)
            nc.vector.memset(zfull[96:97, :, :], 0.0)
            nc.sync.dma_start(zfull[32::64, 0, :], z_ap[:, 2048, :])
            for c in range(2):
                nc.sync.dma_start(
                    zfull[64 * c : 64 * c + 32, :, :], z_ap[c, 0:2048, :]
                )

            for g in range(33):
                ps = s1ps.tile([128, T], f32, tag="s1ps")
                if g in (0, 32):
                    nc.tensor.matmul(
                        ps[:], w1_sb[:, 2 * g, :], zfull[0:97, g, :],
                        start=True, stop=True,
                    )
                else:
                    nc.tensor.matmul(
                        ps[:], w1_sb[:, 2 * g, :], zfull[0:97, g, :],
                        start=True, stop=False,
                    )
                    nc.tensor.matmul(
                        ps[:], w1_sb[:, 2 * g + 1, :], zfull[0:97, 64 - g, :],
                        start=False, stop=True,
                    )
                ab = apool.tile([128, T], bf16, tag="aslot")
                (nc.vector.tensor_copy if g % 3 else nc.scalar.copy)(ab[:], ps[:])
                # SB->SB corner turn: src row (ri*64+m1) -> dst partition
                # {g, 33+g} free block (m1, t)
                (nc.sync if g % 2 else nc.scalar).dma_start(rt[g::33, :, :], ab[:])

        # ---- phase 2: stage 2 + OLA layout turn + imag channel
        ph2 = ctx.enter_context(tc.tile_pool(name="ph2", bufs=1))
        x2 = ph2.tile([128, 32, T], bf16)  # [(par*64+m1), nh, t]
        s2ps = ctx.enter_context(tc.tile_pool(name="s2ps", bufs=3, space="PSUM"))
        xpool = ctx.enter_context(tc.tile_pool(name="xslot", bufs=4))
        impool = ctx.enter_context(tc.tile_pool(name="imps", bufs=2, space="PSUM"))
        imsb = ctx.enter_context(tc.tile_pool(name="imsb", bufs=2))

        def emit_im_block(sc):
            it = imsb.tile([128, 1024], bf16, tag="imsb")
            for half in range(2):
                ips = impool.tile([128, 512], f32, tag="imps")
                nc.tensor.matmul(
                    ips[:],
                    cs[:, sc * 128 : (sc + 1) * 128],
                    wim_sb[:, 512 * half : 512 * (half + 1)],
                    start=True,
                    stop=True,
                )
                nc.any.tensor_copy(it[:, 512 * half : 512 * (half + 1)], ips[:])
            nc.sync.dma_start(outim_ap[sc], it[:])

        im_next = 0
        for m1 in range(64):
            ps2 = s2ps.tile([64, T], f32, tag="s2ps")
            nc.tensor.matmul(
                ps2[:], w2_sb[:, m1, :], rt[:, m1, :], start=True, stop=True
            )
            xs = xpool.tile([64, T], bf16, tag="xslot")
            (nc.vector.tensor_copy if m1 % 3 else nc.scalar.copy)(xs[:], ps2[:])
            # SB->SB layout turn: src row (par*32+nh) -> dst partition
            # {m1, 64+m1} free block (nh, t)
            (nc.sync if m1 % 2 else nc.scalar).dma_start(x2[m1::64, :, :], xs[:])
            if m1 % 12 == 11 and im_next < SC:
                emit_im_block(im_next)
                im_next += 1
        while im_next < SC:
            emit_im_block(im_next)
            im_next += 1

        # ---- OLA (real): sig[p, ih, s] += x2[p, ih + 8r, s - r]
        for r in range(4):
            nc.vector.tensor_add(
                sig[:, :, r : r + T],
                sig[:, :, r : r + T],
                x2[:, 8 * r : 8 * r + 8, :],
            )
        nc.sync.dma_start(outre_ap[:], sig[:])


# ---------------------------------------------------------------- build + run
_CACHE = {}
SPAD = 520  # padded s extent of outre (>= T_CORE + 3)


def _build(T):
    import concourse.bacc as bacc
    import concourse.tile as tile
    import concourse.mybir as mybir

    dt = mybir.dt
    SC = (T + 3 + 127) // 128
    nc = bacc.Bacc("TRN2", target_bir_lowering=False, debug=False, num_devices=N_CORES)
    z_t = nc.dram_tensor("z", [2, FREQ, T], dt.bfloat16, kind="ExternalInput")
    w1_t = nc.dram_tensor("w1", [97, 66, 128], dt.bfloat16, kind="ExternalInput")
    w2_t = nc.dram_tensor("w2", [66, 64, 64], dt.bfloat16, kind="ExternalInput")
    wim_t = nc.dram_tensor("wim", [8, 1024], dt.bfloat16, kind="ExternalInput")
    spad = max(SPAD, T + 3)
    outre_t = nc.dram_tensor("outre", [128, 8, spad], dt.float32, kind="ExternalOutput")
    outim_t = nc.dram_tensor(
        "outim", [SC, 128, 1024], dt.bfloat16, kind="ExternalOutput"
    )
    with tile.TileContext(nc) as tc:
        emit_kernel(
            tc, outre_t.ap(), outim_t.ap(), z_t.ap(), w1_t.ap(), w2_t.ap(),
            wim_t.ap(), T,
        )
    nc.compile()
    return nc


def core_out_to_sig(outre, outim, T):
    """[128,8,spad] f32 + [SC,128,1024] bf16 -> [2, (T+3)*1024] f32."""
    SB = T + 3
    re = outre.transpose(2, 1, 0).reshape(-1, 1024)[:SB]  # [s, i]
    im = np.asarray(outim, dtype=np.float32).reshape(-1, 1024)[:SB]
    return np.stack([re.reshape(-1), im.reshape(-1)])


def make_in_maps(z, window):
    """Shard full f32 inputs into per-core bf16 in_maps."""
    zb = np.asarray(z, dtype=np.float32).astype(_bf16)
    wkey = window.tobytes()
    if _CACHE.get("wkey") != wkey:
        _CACHE["weights"] = build_weights(np.asarray(window, dtype=np.float32))
        _CACHE["wkey"] = wkey
    w1, w2, wim = _CACHE["weights"]
    in_maps = []
    for m in range(N_CORES):
        zc = np.ascontiguousarray(zb[:, :, m * T_CORE : (m + 1) * T_CORE])
        in_maps.append({"z": zc, "w1": w1, "w2": w2, "wim": wim})
    return in_maps


def kernel(z, window):
    from concourse.bass_utils import run_bass_kernel_spmd

    z = np.asarray(z, dtype=np.float32)
    window = np.asarray(window, dtype=np.float32)
    assert z.shape == (2, FREQ, T_FRAMES)

    if "nc" not in _CACHE:
        _CACHE["nc"] = _build(T_CORE)
    nc = _CACHE["nc"]

    in_maps = make_in_maps(z, window)
    res = run_bass_kernel_spmd(nc, in_maps, core_ids=list(range(N_CORES)))

    full = np.zeros((2, L_FULL), dtype=np.float32)
    span = (T_CORE + 3) * 1024
    for m in range(N_CORES):
        o = core_out_to_sig(res.results[m]["outre"], res.results[m]["outim"], T_CORE)
        full[:, m * T_CORE * HOP : m * T_CORE * HOP + span] += o
    out = full[:, N_FFT // 2 : L_FULL - N_FFT // 2]

    win = window.astype(np.float64)
    ws_start = win[0:1024] + win[1024:2048] + win[2048:3072]
    ws_end = win[1024:2048] + win[2048:3072] + win[3072:4096]
    out[:, :1024] *= ((3.0 / 4096.0) / ws_start).astype(np.float32)[None, :]
    out[:, -1024:] *= ((3.0 / 4096.0) / ws_end).astype(np.float32)[None, :]
    return out


# revision 11
# speedup vs baseline: 1.3452x; 1.3452x over previous
"""Trainium2 Bass kernel for CustomISTFT (N_FFT=4096, HOP=1024, T=4096 frames).

Per core (frames sharded 512/core across 8 cores):
  Cooley-Tukey split of the 4096-point inverse DFT: k = 64*j1 + res,
  n = m1 + 64*m2.  z is loaded ONCE into SBUF as zfull[(c,j1), res, t]
  (two big k-major DMAs), so stage 1 needs no gathers: column c of the
  residue DFT is two accumulating matmuls (res=c and res=64-c) against
  zfull slices, contraction over (channel, j1) = 97 partitions.
  Hermitian symmetry gives A[m1, 64-c] = conj(A[m1, c]) so only
  c = 0..32 is computed; the conjugate fold, window and normalization
  live in the stage-2 weights (contraction over (re/im, c) = 66).
  Both corner turns (m1 <-> c and m2 -> OLA layout) are DRAM round
  trips done as a few LARGE DMAs (>=1024 descriptors each so the HWDGE
  fans them out across all 16 DMA engines; small-descriptor-count DMAs
  run on a single engine).  DMA triggers alternate between the two
  HWDGE queues (sync + scalar).  The overlap-add runs on the vector
  engine in [n mod 128, n div 128, t] layout, split into partition
  halves so it overlaps the x readback.  The imaginary channel is
  win[n]*(b0[t] + (-1)^n b2048[t])/4096 (rank-2 per parity) via K=8
  matmuls that also perform its overlap-add; it ships as one bf16
  tile.  z is pre-cast to bf16 on the host.  Host: shard, gather,
  reorder, halo-add between neighbor cores, exact wsum correction on
  the two edge blocks.
"""

import numpy as np
import ml_dtypes

N_FFT = 4096
HOP = 1024
FREQ = 2049
T_FRAMES = 4096
N_CORES = 8
T_CORE = T_FRAMES // N_CORES  # 512
L_FULL = (T_FRAMES - 1) * HOP + N_FFT
OUT_LEN = L_FULL - N_FFT

_bf16 = ml_dtypes.bfloat16


# ---------------------------------------------------------------- weights
def build_weights(window):
    """w1 [97,66,128] bf16: row (c*64+j1) matches zfull partition, col
    2g+half (half 0: res=g, half 1: res=64-g), inner (ri_out, m1).
    w2 [66,64,64] bf16 ([(ri*33+c), m1, (par,nh)-col], conj-fold +
    window/3 folded).  wim [8,1024] bf16."""
    win = window.astype(np.float64)
    mu = np.exp(2j * np.pi / 4096)
    w64c = np.exp(2j * np.pi / 64)
    m1v = np.arange(64)

    w1 = np.zeros((97, 66, 128), dtype=np.float64)
    for g in range(33):
        coef = {}
        for j1 in range(64):
            k = 64 * j1 + g
            e = w64c ** (m1v * j1)
            if k <= 2048:
                coef[(0, k)] = coef.get((0, k), 0) + e
                coef[(1, k)] = coef.get((1, k), 0) + 1j * e
            else:
                kr = 4096 - k
                coef[(0, kr)] = coef.get((0, kr), 0) + e
                coef[(1, kr)] = coef.get((1, kr), 0) - 1j * e
        tw = mu ** (m1v * g)
        for (ch, k), v0 in coef.items():
            v = v0 * tw
            res, j1 = k % 64, k // 64
            if g in (0, 32):
                col = 2 * g
            elif res == g:
                col = 2 * g
            else:
                assert res == 64 - g
                col = 2 * g + 1
            row = ch * 64 + j1
            w1[row, col, :64] += v.real
            w1[row, col, 64:] += v.imag

    # stage 2: x[m1 + 64*m2] = (1/4096) * [ ReA[m1,0] + (-1)^m2 ReA[m1,32]
    #   + sum_{c=1..31} 2*(cos(th) ReA[m1,c] - sin(th) ImA[m1,c]) ],
    # th = 2*pi*m2*c/64, then * win[n]*4096/3.  Column cc = par*32+nh
    # with m2 = 2*nh+par.
    w2 = np.zeros((66, 64, 64), dtype=np.float64)
    cc = np.arange(64)
    m2 = 2 * (cc % 32) + (cc // 32)
    for c in range(33):
        fac = 2.0 if 1 <= c <= 31 else 1.0
        th = 2 * np.pi * m2 * c / 64.0
        for m1 in range(64):
            n = m1 + 64 * m2
            wn = win[n] / 3.0  # win * (4096/3) / 4096
            w2[c, m1, :] = fac * np.cos(th) * wn
            w2[33 + c, m1, :] = -fac * np.sin(th) * wn

    # wim[(2r+par), i] = win[i + 1024 r]/3 * (par == i%2)
    wim = np.zeros((8, 1024), dtype=np.float64)
    iv = np.arange(1024)
    for r in range(4):
        for par in range(2):
            wim[2 * r + par] = (win[iv + 1024 * r] / 3.0) * (iv % 2 == par)
    return w1.astype(_bf16), w2.astype(_bf16), wim.astype(_bf16)


# ---------------------------------------------------------------- device program
def emit_kernel(tc, outre_ap, outim_ap, z_ap, w1_ap, w2_ap, wim_ap, T):
    """Per-core program.  T frames (multiple of 128).
    outre [128, 8, SPAD] f32:  outre[p, ih, s] =
        sum_r win*x[p + 128*ih + 1024*r, s - r]  (real channel, s in [0,T+3))
    outim [128, SC, 1024] bf16: outim[sp, sc, i] = imag channel at block
        s = 128*sc + sp, position i."""
    import concourse.mybir as mybir
    from contextlib import ExitStack

    nc = tc.nc
    dt = mybir.dt
    f32, bf16 = dt.float32, dt.bfloat16
    SB = T + 3
    SC = (SB + 127) // 128
    SPAD = outre_ap.shape[2]
    assert SPAD >= SB and outim_ap.shape[1] == SC

    with ExitStack() as ctx:
        const = ctx.enter_context(tc.tile_pool(name="const", bufs=1))
        dram = ctx.enter_context(tc.tile_pool(name="dram", bufs=1, space="DRAM"))

        # persistent tiles
        sig = const.tile([128, 8, SPAD], f32)
        w2_sb = const.tile([66, 64, 64], bf16)
        wim_sb = const.tile([8, 1024], bf16)
        cve = const.tile([1, T], bf16)
        cvo = const.tile([1, T], bf16)
        b0t = const.tile([1, T], bf16)
        b2t = const.tile([1, T], bf16)
        cs = const.tile([8, SC * 128], bf16)
        imall = const.tile([128, SC, 1024], bf16)

        a_dram = dram.tile([128, 33, T], bf16)  # [(ri,m1), g, t]
        x_dram = dram.tile([64, 64, T], bf16)  # [(par,nh), m1, t]

        nc.scalar.dma_start(w2_sb[:], w2_ap[:], max_dma_last_dim=256)
        nc.scalar.dma_start(wim_sb[:], wim_ap[:])
        nc.any.memset(sig[:], 0.0)
        nc.any.memset(cs[:], 0.0)

        # b0 = z[1,0,:], b2048 = z[1,2048,:];  CS[2r+par, s] = cv_par[s-r]
        nc.sync.dma_start(b0t[:], z_ap[1, 0:1, :])
        nc.sync.dma_start(b2t[:], z_ap[1, 2048:2049, :])
        nc.vector.tensor_add(cve[:], b0t[:], b2t[:])
        nc.vector.tensor_sub(cvo[:], b0t[:], b2t[:])
        for r in range(4):
            nc.sync.dma_start(cs[2 * r : 2 * r + 1, r : r + T], cve[:])
            nc.sync.dma_start(cs[2 * r + 1 : 2 * r + 2, r : r + T], cvo[:])

        # ---- phase 1: z resident in SBUF, 33 stage-1 calls, A to DRAM
        with (
            tc.tile_pool(name="ph1", bufs=1) as ph1,
            tc.tile_pool(name="s1ps", bufs=3, space="PSUM") as s1ps,
            tc.tile_pool(name="achunk", bufs=2) as apool,
        ):
            # zfull[(c*64+j1), res, t] = z[c, res + 64*j1, t]; rows 33-63
            # are zero pad (weights there are zero), row 32/96 only res=0
            # is real data (k=2048), other res zeroed.
            zfull = ph1.tile([128, 64, T], bf16)
            w1_sb = ph1.tile([97, 66, 128], bf16)
            nc.scalar.dma_start(w1_sb[:], w1_ap[:], max_dma_last_dim=384)
            nc.vector.memset(zfull[32:64, :, :], 0.0)
            nc.vector.memset(zfull[96:97, :, :], 0.0)
            nc.sync.dma_start(zfull[32::64, 0, :], z_ap[:, 2048, :])
            nc.sync.dma_start(zfull[0:32, :, :], z_ap[0, 0:2048, :])
            nc.scalar.dma_start(zfull[64:96, :, :], z_ap[1, 0:2048, :])

            CH = 8  # stage-1 groups per A-chunk
            ab = None
            for g in range(33):
                ps = s1ps.tile([128, T], f32, tag="s1ps")
                if g in (0, 32):
                    nc.tensor.matmul(
                        ps[:], w1_sb[:, 2 * g, :], zfull[0:97, g, :],
                        start=True, stop=True,
                    )
                else:
                    nc.tensor.matmul(
                        ps[:], w1_sb[:, 2 * g, :], zfull[0:97, g, :],
                        start=True, stop=False,
                    )
                    nc.tensor.matmul(
                        ps[:], w1_sb[:, 2 * g + 1, :], zfull[0:97, 64 - g, :],
                        start=False, stop=True,
                    )
                gi = g % CH
                if gi == 0:
                    ab = apool.tile([128, CH, T], bf16, tag="achunk")
                (nc.vector.tensor_copy if g % 3 else nc.scalar.copy)(
                    ab[:, gi, :], ps[:]
                )
                if gi == CH - 1 or g == 32:
                    g0 = g - gi
                    (nc.sync if (g // CH) % 2 else nc.scalar).dma_start(
                        a_dram[:, g0 : g + 1, :], ab[:, 0 : gi + 1, :]
                    )

        # ---- phase 2: corner-turn readback, stage 2, x to DRAM, imag
        ph2 = ctx.enter_context(tc.tile_pool(name="ph2", bufs=1))
        rt = ph2.tile([66, 64, T], bf16)  # A^T: [(ri*33+c), m1, t]
        x2 = ph2.tile([128, 32, T], bf16)  # [(par*64+m1), nh, t]
        # rt[ri*33+g, m1, t] = a_dram[ri*64+m1, g, t]
        for ri in range(2):
            src = a_dram[:].rearrange("(ri m) g t -> ri g m t", ri=2)[ri]
            (nc.sync if ri else nc.scalar).dma_start(
                rt[33 * ri : 33 * ri + 33, :, :], src
            )

        s2ps = ctx.enter_context(tc.tile_pool(name="s2ps", bufs=3, space="PSUM"))
        xpool = ctx.enter_context(tc.tile_pool(name="xchunk", bufs=2))
        impool = ctx.enter_context(tc.tile_pool(name="imps", bufs=2, space="PSUM"))

        def emit_im_block(sc):
            for half in range(2):
                ips = impool.tile([128, 512], f32, tag="imps")
                nc.tensor.matmul(
                    ips[:],
                    cs[:, sc * 128 : (sc + 1) * 128],
                    wim_sb[:, 512 * half : 512 * (half + 1)],
                    start=True,
                    stop=True,
                )
                nc.any.tensor_copy(
                    imall[:, sc, 512 * half : 512 * (half + 1)], ips[:]
                )

        XH = 16  # stage-2 m1 per x-chunk
        im_next = 0
        xc = None
        for m1 in range(64):
            ps2 = s2ps.tile([64, T], f32, tag="s2ps")
            nc.tensor.matmul(
                ps2[:], w2_sb[:, m1, :], rt[:, m1, :], start=True, stop=True
            )
            mi = m1 % XH
            if mi == 0:
                xc = xpool.tile([64, XH, T], bf16, tag="xchunk")
            (nc.vector.tensor_copy if m1 % 3 else nc.scalar.copy)(
                xc[:, mi, :], ps2[:]
            )
            if mi == XH - 1:
                m0 = m1 - mi
                (nc.sync if (m1 // XH) % 2 else nc.scalar).dma_start(
                    x_dram[:, m0 : m1 + 1, :], xc[:]
                )
            if m1 % 12 == 11 and im_next < SC:
                emit_im_block(im_next)
                im_next += 1
        while im_next < SC:
            emit_im_block(im_next)
            im_next += 1
        nc.scalar.dma_start(outim_ap[:], imall[:], max_dma_last_dim=512)

        # x2[par*64+m1, nh, t] = x_dram[par*32+nh, m1, t]; then per-half OLA
        for par in range(2):
            src = x_dram[:].rearrange("(par nh) m t -> par m nh t", par=2)[par]
            (nc.sync if par else nc.scalar).dma_start(
                x2[64 * par : 64 * par + 64, :, :], src
            )
            for r in range(4):
                nc.vector.tensor_add(
                    sig[64 * par : 64 * par + 64, :, r : r + T],
                    sig[64 * par : 64 * par + 64, :, r : r + T],
                    x2[64 * par : 64 * par + 64, 8 * r : 8 * r + 8, :],
                )
            (nc.scalar if par else nc.sync).dma_start(
                outre_ap[64 * par : 64 * par + 64, :, :],
                sig[64 * par : 64 * par + 64, :, :],
                max_dma_last_dim=260,
            )


# ---------------------------------------------------------------- build + run
_CACHE = {}
SPAD = 520  # padded s extent of outre (>= T_CORE + 3)


def _build(T):
    import concourse.bacc as bacc
    import concourse.tile as tile
    import concourse.mybir as mybir

    dt = mybir.dt
    SC = (T + 3 + 127) // 128
    nc = bacc.Bacc("TRN2", target_bir_lowering=False, debug=False, num_devices=N_CORES)
    z_t = nc.dram_tensor("z", [2, FREQ, T], dt.bfloat16, kind="ExternalInput")
    w1_t = nc.dram_tensor("w1", [97, 66, 128], dt.bfloat16, kind="ExternalInput")
    w2_t = nc.dram_tensor("w2", [66, 64, 64], dt.bfloat16, kind="ExternalInput")
    wim_t = nc.dram_tensor("wim", [8, 1024], dt.bfloat16, kind="ExternalInput")
    spad = max(SPAD, T + 3)
    outre_t = nc.dram_tensor("outre", [128, 8, spad], dt.float32, kind="ExternalOutput")
    outim_t = nc.dram_tensor(
        "outim", [128, SC, 1024], dt.bfloat16, kind="ExternalOutput"
    )
    with tile.TileContext(nc) as tc:
        emit_kernel(
            tc, outre_t.ap(), outim_t.ap(), z_t.ap(), w1_t.ap(), w2_t.ap(),
            wim_t.ap(), T,
        )
    nc.compile()
    return nc


def core_out_to_sig(outre, outim, T):
    """[128,8,spad] f32 + [128,SC,1024] bf16 -> [2, (T+3)*1024] f32."""
    SB = T + 3
    re = outre.transpose(2, 1, 0).reshape(-1, 1024)[:SB]  # [s, i]
    im = (
        np.asarray(outim, dtype=np.float32)
        .transpose(1, 0, 2)
        .reshape(-1, 1024)[:SB]
    )
    return np.stack([re.reshape(-1), im.reshape(-1)])


def make_in_maps(z, window):
    """Shard full f32 inputs into per-core bf16 in_maps."""
    zb = np.asarray(z, dtype=np.float32).astype(_bf16)
    wkey = window.tobytes()
    if _CACHE.get("wkey") != wkey:
        _CACHE["weights"] = build_weights(np.asarray(window, dtype=np.float32))
        _CACHE["wkey"] = wkey
    w1, w2, wim = _CACHE["weights"]
    in_maps = []
    for m in range(N_CORES):
        zc = np.ascontiguousarray(zb[:, :, m * T_CORE : (m + 1) * T_CORE])
        in_maps.append({"z": zc, "w1": w1, "w2": w2, "wim": wim})
    return in_maps


def kernel(z, window):
    from concourse.bass_utils import run_bass_kernel_spmd

    z = np.asarray(z, dtype=np.float32)
    window = np.asarray(window, dtype=np.float32)
    assert z.shape == (2, FREQ, T_FRAMES)

    if "nc" not in _CACHE:
        _CACHE["nc"] = _build(T_CORE)
    nc = _CACHE["nc"]

    in_maps = make_in_maps(z, window)
    res = run_bass_kernel_spmd(nc, in_maps, core_ids=list(range(N_CORES)))

    full = np.zeros((2, L_FULL), dtype=np.float32)
    span = (T_CORE + 3) * 1024
    for m in range(N_CORES):
        o = core_out_to_sig(res.results[m]["outre"], res.results[m]["outim"], T_CORE)
        full[:, m * T_CORE * HOP : m * T_CORE * HOP + span] += o
    out = full[:, N_FFT // 2 : L_FULL - N_FFT // 2]

    win = window.astype(np.float64)
    ws_start = win[0:1024] + win[1024:2048] + win[2048:3072]
    ws_end = win[1024:2048] + win[2048:3072] + win[3072:4096]
    out[:, :1024] *= ((3.0 / 4096.0) / ws_start).astype(np.float32)[None, :]
    out[:, -1024:] *= ((3.0 / 4096.0) / ws_end).astype(np.float32)[None, :]
    return out


# revision 12
# speedup vs baseline: 1.7092x; 1.2706x over previous
"""Trainium2 Bass kernel for CustomISTFT (N_FFT=4096, HOP=1024, T=4096 frames).

Per core (frames sharded 512/core across 8 cores):
  Cooley-Tukey split of the 4096-point inverse DFT: k = 64*j1 + res,
  n = m1 + 64*m2.  z is loaded ONCE into SBUF as zfull[(c,j1), res, t]
  (two big k-major DMAs), so stage 1 needs no gathers: column c of the
  residue DFT is two accumulating matmuls (res=c and res=64-c) against
  zfull slices, contraction over (channel, j1) = 97 partitions.
  Hermitian symmetry gives A[m1, 64-c] = conj(A[m1, c]) so only
  c = 0..32 is computed; the conjugate fold, window and normalization
  live in the stage-2 weights (contraction over (re/im, c) = 66).
  Both corner turns (m1 <-> c and m2 -> OLA layout) are DRAM round
  trips done as a few LARGE DMAs (>=1024 descriptors each so the HWDGE
  fans them out across all 16 DMA engines; small-descriptor-count DMAs
  run on a single engine).  DMA triggers alternate between the two
  HWDGE queues (sync + scalar).  The overlap-add runs on the vector
  engine in [n mod 128, n div 128, t] layout, split into partition
  halves so it overlaps the x readback.  The imaginary channel is
  win[n]*(b0[t] + (-1)^n b2048[t])/4096 (rank-2 per parity) via K=8
  matmuls that also perform its overlap-add; it ships as one bf16
  tile.  z is pre-cast to bf16 on the host.  Host: shard, gather,
  reorder, halo-add between neighbor cores, exact wsum correction on
  the two edge blocks.
"""

import numpy as np
import ml_dtypes

N_FFT = 4096
HOP = 1024
FREQ = 2049
T_FRAMES = 4096
N_CORES = 8
T_CORE = T_FRAMES // N_CORES  # 512
L_FULL = (T_FRAMES - 1) * HOP + N_FFT
OUT_LEN = L_FULL - N_FFT

_bf16 = ml_dtypes.bfloat16


# ---------------------------------------------------------------- weights
def build_weights(window):
    """w1 [97,66,128] bf16: row (c*64+j1) matches zfull partition, col
    2g+half (half 0: res=g, half 1: res=64-g), inner (ri_out, m1).
    w2 [66,64,64] bf16 ([(ri*33+c), m1, (par,nh)-col], conj-fold +
    window/3 folded).  wim [8,1024] bf16."""
    win = window.astype(np.float64)
    mu = np.exp(2j * np.pi / 4096)
    w64c = np.exp(2j * np.pi / 64)
    m1v = np.arange(64)

    w1 = np.zeros((128, 66, 128), dtype=np.float64)
    for g in range(33):
        coef = {}
        for j1 in range(64):
            k = 64 * j1 + g
            e = w64c ** (m1v * j1)
            if k <= 2048:
                coef[(0, k)] = coef.get((0, k), 0) + e
                coef[(1, k)] = coef.get((1, k), 0) + 1j * e
            else:
                kr = 4096 - k
                coef[(0, kr)] = coef.get((0, kr), 0) + e
                coef[(1, kr)] = coef.get((1, kr), 0) - 1j * e
        tw = mu ** (m1v * g)
        for (ch, k), v0 in coef.items():
            v = v0 * tw
            res, j1 = k % 64, k // 64
            if g in (0, 32):
                col = 2 * g
            elif res == g:
                col = 2 * g
            else:
                assert res == 64 - g
                col = 2 * g + 1
            row = ch * 64 + j1
            w1[row, col, :64] += v.real
            w1[row, col, 64:] += v.imag

    # stage 2: x[m1 + 64*m2] = (1/4096) * [ ReA[m1,0] + (-1)^m2 ReA[m1,32]
    #   + sum_{c=1..31} 2*(cos(th) ReA[m1,c] - sin(th) ImA[m1,c]) ],
    # th = 2*pi*m2*c/64, then * win[n]*4096/3.  Column cc = par*32+nh
    # with m2 = 2*nh+par.
    w2 = np.zeros((80, 64, 64), dtype=np.float64)
    cc = np.arange(64)
    m2 = 2 * (cc % 32) + (cc // 32)
    for c in range(33):
        fac = 2.0 if 1 <= c <= 31 else 1.0
        th = 2 * np.pi * m2 * c / 64.0
        for m1 in range(64):
            n = m1 + 64 * m2
            wn = win[n] / 3.0  # win * (4096/3) / 4096
            w2[c, m1, :] = fac * np.cos(th) * wn
            w2[33 + c, m1, :] = -fac * np.sin(th) * wn

    # wim[(2r+par), i] = win[i + 1024 r]/3 * (par == i%2)
    wim = np.zeros((8, 1024), dtype=np.float64)
    iv = np.arange(1024)
    for r in range(4):
        for par in range(2):
            wim[2 * r + par] = (win[iv + 1024 * r] / 3.0) * (iv % 2 == par)
    return w1.astype(_bf16), w2.astype(_bf16), wim.astype(_bf16)


# ---------------------------------------------------------------- device program
def emit_kernel(tc, outre_ap, outim_ap, z_ap, w1_ap, w2_ap, wim_ap, T):
    """Per-core program.  T frames (multiple of 128).
    outre [128, 8, SPAD] f32:  outre[p, ih, s] =
        sum_r win*x[p + 128*ih + 1024*r, s - r]  (real channel, s in [0,T+3))
    outim [128, SC, 1024] bf16: outim[sp, sc, i] = imag channel at block
        s = 128*sc + sp, position i."""
    import concourse.mybir as mybir
    from contextlib import ExitStack

    nc = tc.nc
    dt = mybir.dt
    f32, bf16 = dt.float32, dt.bfloat16
    SB = T + 3
    SC = (SB + 127) // 128
    SPAD = outre_ap.shape[2]
    assert SPAD >= SB and outim_ap.shape[1] == SC

    with ExitStack() as ctx:
        const = ctx.enter_context(tc.tile_pool(name="const", bufs=1))
        dram = ctx.enter_context(tc.tile_pool(name="dram", bufs=1, space="DRAM"))

        # persistent tiles
        sig = const.tile([128, 8, SPAD], bf16)
        w2_sb = const.tile([80, 64, 64], bf16)
        wim_sb = const.tile([8, 1024], bf16)
        cve = const.tile([1, T], bf16)
        cvo = const.tile([1, T], bf16)
        b0t = const.tile([1, T], bf16)
        b2t = const.tile([1, T], bf16)
        cs = const.tile([8, SC * 128], bf16)
        imall = const.tile([128, SC, 1024], bf16)

        a_dram = dram.tile([128, 33, T], bf16)  # [(ri,m1), g, t]
        x_dram = dram.tile([64, 64, T], bf16)  # [(par,nh), m1, t]

        nc.any.memset(sig[:], 0.0)
        nc.any.memset(cs[:], 0.0)

        # b0 = z[1,0,:], b2048 = z[1,2048,:];  CS[2r+par, s] = cv_par[s-r]
        nc.sync.dma_start(b0t[:], z_ap[1, 0:1, :])
        nc.sync.dma_start(b2t[:], z_ap[1, 2048:2049, :])
        nc.vector.tensor_add(cve[:], b0t[:], b2t[:])
        nc.vector.tensor_sub(cvo[:], b0t[:], b2t[:])
        for r in range(4):
            nc.sync.dma_start(cs[2 * r : 2 * r + 1, r : r + T], cve[:])
            nc.sync.dma_start(cs[2 * r + 1 : 2 * r + 2, r : r + T], cvo[:])

        # ---- phase 1: z resident in SBUF, 33 stage-1 calls, A to DRAM
        with (
            tc.tile_pool(name="ph1", bufs=1) as ph1,
            tc.tile_pool(name="s1ps", bufs=3, space="PSUM") as s1ps,
            tc.tile_pool(name="achunk", bufs=2) as apool,
        ):
            # zfull[(c*64+j1), res, t] = z[c, res + 64*j1, t]; rows 33-63
            # are zero pad (weights there are zero), row 32/96 only res=0
            # is real data (k=2048), other res zeroed.
            zfull = ph1.tile([128, 64, T], bf16)
            w1_sb = ph1.tile([128, 66, 128], bf16)
            nc.scalar.dma_start(w1_sb[:], w1_ap[:])
            nc.vector.memset(zfull[32:64, :, :], 0.0)
            nc.vector.memset(zfull[96:97, :, :], 0.0)
            nc.sync.dma_start(zfull[32::64, 0, :], z_ap[:, 2048, :])
            nc.sync.dma_start(zfull[0:32, :, :], z_ap[0, 0:2048, :])
            nc.scalar.dma_start(zfull[64:96, :, :], z_ap[1, 0:2048, :])
            nc.scalar.dma_start(w2_sb[:], w2_ap[:])
            nc.scalar.dma_start(wim_sb[:], wim_ap[:])

            CH = 8  # stage-1 groups per A-chunk
            ab = None
            for g in range(33):
                ps = s1ps.tile([128, T], f32, tag="s1ps")
                if g in (0, 32):
                    nc.tensor.matmul(
                        ps[:], w1_sb[0:97, 2 * g, :], zfull[0:97, g, :],
                        start=True, stop=True,
                    )
                else:
                    nc.tensor.matmul(
                        ps[:], w1_sb[0:97, 2 * g, :], zfull[0:97, g, :],
                        start=True, stop=False,
                    )
                    nc.tensor.matmul(
                        ps[:], w1_sb[0:97, 2 * g + 1, :], zfull[0:97, 64 - g, :],
                        start=False, stop=True,
                    )
                gi = g % CH
                if gi == 0:
                    ab = apool.tile([128, CH, T], bf16, tag="achunk")
                (nc.vector.tensor_copy if g % 3 else nc.scalar.copy)(
                    ab[:, gi, :], ps[:]
                )
                if gi == CH - 1 or g == 32:
                    g0 = g - gi
                    (nc.sync if (g // CH) % 2 else nc.scalar).dma_start(
                        a_dram[:, g0 : g + 1, :], ab[:, 0 : gi + 1, :]
                    )

        # ---- phase 2: corner-turn readback, stage 2, x to DRAM, imag
        ph2 = ctx.enter_context(tc.tile_pool(name="ph2", bufs=1))
        rt = ph2.tile([66, 64, T], bf16)  # A^T: [(ri*33+c), m1, t]
        x2 = ph2.tile([128, 32, T], bf16)  # [(par*64+m1), nh, t]
        # rt[ri*33+g, m1, t] = a_dram[ri*64+m1, g, t]
        for ri in range(2):
            src = a_dram[:].rearrange("(ri m) g t -> ri g m t", ri=2)[ri]
            (nc.sync if ri else nc.scalar).dma_start(
                rt[33 * ri : 33 * ri + 33, :, :], src
            )

        s2ps = ctx.enter_context(tc.tile_pool(name="s2ps", bufs=3, space="PSUM"))
        xpool = ctx.enter_context(tc.tile_pool(name="xchunk", bufs=2))
        impool = ctx.enter_context(tc.tile_pool(name="imps", bufs=2, space="PSUM"))

        def emit_im_block(sc):
            for half in range(2):
                ips = impool.tile([128, 512], f32, tag="imps")
                nc.tensor.matmul(
                    ips[:],
                    cs[:, sc * 128 : (sc + 1) * 128],
                    wim_sb[:, 512 * half : 512 * (half + 1)],
                    start=True,
                    stop=True,
                )
                nc.any.tensor_copy(
                    imall[:, sc, 512 * half : 512 * (half + 1)], ips[:]
                )

        XH = 16  # stage-2 m1 per x-chunk
        im_next = 0
        xc = None
        for m1 in range(64):
            ps2 = s2ps.tile([64, T], f32, tag="s2ps")
            nc.tensor.matmul(
                ps2[:], w2_sb[0:66, m1, :], rt[:, m1, :], start=True, stop=True
            )
            mi = m1 % XH
            if mi == 0:
                xc = xpool.tile([64, XH, T], bf16, tag="xchunk")
            (nc.vector.tensor_copy if m1 % 3 else nc.scalar.copy)(
                xc[:, mi, :], ps2[:]
            )
            if mi == XH - 1:
                m0 = m1 - mi
                (nc.sync if (m1 // XH) % 2 else nc.scalar).dma_start(
                    x_dram[:, m0 : m1 + 1, :], xc[:]
                )
            if m1 % 12 == 11 and im_next < SC:
                emit_im_block(im_next)
                im_next += 1
        while im_next < SC:
            emit_im_block(im_next)
            im_next += 1
        nc.scalar.dma_start(outim_ap[:], imall[:], max_dma_last_dim=512)

        # x2[par*64+m1, nh, t] = x_dram[par*32+nh, m1, t]; then OLA
        for par in range(2):
            src = x_dram[:].rearrange("(par nh) m t -> par m nh t", par=2)[par]
            (nc.sync if par else nc.scalar).dma_start(
                x2[64 * par : 64 * par + 64, :, :], src
            )
        for r in range(4):
            nc.vector.tensor_add(
                sig[:, :, r : r + T],
                sig[:, :, r : r + T],
                x2[:, 8 * r : 8 * r + 8, :],
            )
        nc.sync.dma_start(outre_ap[:], sig[:], max_dma_last_dim=260)


# ---------------------------------------------------------------- build + run
_CACHE = {}
SPAD = 520  # padded s extent of outre (>= T_CORE + 3)


def _build(T):
    import concourse.bacc as bacc
    import concourse.tile as tile
    import concourse.mybir as mybir

    dt = mybir.dt
    SC = (T + 3 + 127) // 128
    nc = bacc.Bacc("TRN2", target_bir_lowering=False, debug=False, num_devices=N_CORES)
    z_t = nc.dram_tensor("z", [2, FREQ, T], dt.bfloat16, kind="ExternalInput")
    w1_t = nc.dram_tensor("w1", [128, 66, 128], dt.bfloat16, kind="ExternalInput")
    w2_t = nc.dram_tensor("w2", [80, 64, 64], dt.bfloat16, kind="ExternalInput")
    wim_t = nc.dram_tensor("wim", [8, 1024], dt.bfloat16, kind="ExternalInput")
    spad = max(SPAD, T + 3)
    outre_t = nc.dram_tensor("outre", [128, 8, spad], dt.bfloat16, kind="ExternalOutput")
    outim_t = nc.dram_tensor(
        "outim", [128, SC, 1024], dt.bfloat16, kind="ExternalOutput"
    )
    with tile.TileContext(nc) as tc:
        emit_kernel(
            tc, outre_t.ap(), outim_t.ap(), z_t.ap(), w1_t.ap(), w2_t.ap(),
            wim_t.ap(), T,
        )
    nc.compile()
    return nc


def core_out_to_sig(outre, outim, T):
    """[128,8,spad] bf16 + [128,SC,1024] bf16 -> [2, (T+3)*1024] f32."""
    SB = T + 3
    re = np.asarray(outre, dtype=np.float32).transpose(2, 1, 0).reshape(-1, 1024)[:SB]  # [s, i]
    im = (
        np.asarray(outim, dtype=np.float32)
        .transpose(1, 0, 2)
        .reshape(-1, 1024)[:SB]
    )
    return np.stack([re.reshape(-1), im.reshape(-1)])


def make_in_maps(z, window):
    """Shard full f32 inputs into per-core bf16 in_maps."""
    zb = np.asarray(z, dtype=np.float32).astype(_bf16)
    wkey = window.tobytes()
    if _CACHE.get("wkey") != wkey:
        _CACHE["weights"] = build_weights(np.asarray(window, dtype=np.float32))
        _CACHE["wkey"] = wkey
    w1, w2, wim = _CACHE["weights"]
    in_maps = []
    for m in range(N_CORES):
        zc = np.ascontiguousarray(zb[:, :, m * T_CORE : (m + 1) * T_CORE])
        in_maps.append({"z": zc, "w1": w1, "w2": w2, "wim": wim})
    return in_maps


def kernel(z, window):
    from concourse.bass_utils import run_bass_kernel_spmd

    z = np.asarray(z, dtype=np.float32)
    window = np.asarray(window, dtype=np.float32)
    assert z.shape == (2, FREQ, T_FRAMES)

    if "nc" not in _CACHE:
        _CACHE["nc"] = _build(T_CORE)
    nc = _CACHE["nc"]

    in_maps = make_in_maps(z, window)
    res = run_bass_kernel_spmd(nc, in_maps, core_ids=list(range(N_CORES)))

    full = np.zeros((2, L_FULL), dtype=np.float32)
    span = (T_CORE + 3) * 1024
    for m in range(N_CORES):
        o = core_out_to_sig(res.results[m]["outre"], res.results[m]["outim"], T_CORE)
        full[:, m * T_CORE * HOP : m * T_CORE * HOP + span] += o
    out = full[:, N_FFT // 2 : L_FULL - N_FFT // 2]

    win = window.astype(np.float64)
    ws_start = win[0:1024] + win[1024:2048] + win[2048:3072]
    ws_end = win[1024:2048] + win[2048:3072] + win[3072:4096]
    out[:, :1024] *= ((3.0 / 4096.0) / ws_start).astype(np.float32)[None, :]
    out[:, -1024:] *= ((3.0 / 4096.0) / ws_end).astype(np.float32)[None, :]
    return out
